# revision 28
# baseline (speedup 1.0000x reference)
"""AttnLSTMDecoder Trainium2 kernel (v2).

Data-parallel over batch: 8 NeuronCores x 8 batches each. The T=64
recurrence runs locally per core; no collectives.

v2 changes vs v1:
  - x-hoist: x_t @ W_x^T precomputed on host for all t (GX); the
    per-step gates matmul only contracts over [prev_out | h] (12 kc
    instead of 16), seeded into PSUM via an identity matmul on GX.
  - Sorted batch assignment: batches sorted by source_length, rank r
    -> (core r%8, slot r//8), so the shared-NEFF per-slot max s-chunk
    counts shrink (63 -> 53 chunks): less SBUF, less encC streaming,
    fewer score/context columns.
  - Coalesced DMA: weights stream as [128, 4096] 1MB tiles (4 kc per
    tile), encC as 2 DMAs per batch from a packed DRAM image.
"""

import os
import sys
from contextlib import ExitStack

import numpy as np

sys.path.insert(0, "/opt/trn_rl_repo")

import ml_dtypes  # noqa: E402

import concourse.bass as bass  # noqa: E402
import concourse.mybir as mybir  # noqa: E402
import concourse.tile as tile  # noqa: E402
import json as _json  # noqa: E402

import concourse.bass_utils as _bu  # noqa: E402
import concourse.bass2jax as _b2j  # noqa: E402
from concourse.bass_utils import run_bass_kernel_spmd  # noqa: E402

_orig_compile_bir_kernel = _bu.compile_bir_kernel


def _strip_ring_waits(bir_str):
    """Work around walrus per-instruction sem-wait limits.

    - DMACopy allows 1 wait: drop DMA ring-throttle waits (redundant with the
      slot-release engine wait; HW rings are deep vs our <=10 in-flight).
    - Engine instrs allow 2: hoist excess waits into a standalone
      EventSemaphore on the same engine directly before the instruction.
    """
    bir = _json.loads(bir_str)
    counter = [0]

    def fix_dma(inst):
        si = inst.get("sync_info")
        if not si:
            return
        ow = si.get("on_wait") or []
        if len(ow) <= 1:
            return
        eng = [w for w in ow if not w.get("ant_name", "").startswith(("DMAHW", "DMASW"))]
        si["on_wait"] = eng[:1] if eng else ow[:1]

    def walk(o):
        if isinstance(o, dict):
            for k, v in o.items():
                if (
                    isinstance(v, list)
                    and v
                    and isinstance(v[0], dict)
                    and "opcode" in v[0]
                ):
                    new = []
                    for inst in v:
                        if inst.get("opcode") == "DMACopy":
                            fix_dma(inst)
                        else:
                            si = inst.get("sync_info") or {}
                            ow = si.get("on_wait") or []
                            if len(ow) > 1:
                                for w in ow[:-1]:
                                    counter[0] += 1
                                    new.append({
                                        "debug": inst.get("debug", 0),
                                        "engine": inst["engine"],
                                        "ins": [],
                                        "name": f"hoist_wait_{counter[0]}",
                                        "opcode": "EventSemaphore",
                                        "outs": [],
                                        "sync_info": {
                                            "on_update": [],
                                            "on_wait": [w],
                                        },
                                    })
                                si["on_wait"] = ow[-1:]
                        new.append(inst)
                        walk(inst)
                    o[k] = new
                elif isinstance(v, (dict, list)):
                    walk(v)
        elif isinstance(o, list):
            for v in o:
                walk(v)

    walk(bir)
    return _json.dumps(bir)


def _patched_compile_bir_kernel(ant_bir_str, *a, **k):
    if isinstance(ant_bir_str, bytes):
        fixed = _strip_ring_waits(ant_bir_str.decode()).encode()
    else:
        fixed = _strip_ring_waits(ant_bir_str)
    return _orig_compile_bir_kernel(fixed, *a, **k)


_bu.compile_bir_kernel = _patched_compile_bir_kernel
_b2j.compile_bir_kernel = _patched_compile_bir_kernel
from concourse.masks import make_identity  # noqa: E402

BF16 = mybir.dt.bfloat16
F32 = mybir.dt.float32

B_FULL, S, T_FULL = 64, 1024, 64
H = 1024
D = 512
NCORES = 8
B = B_FULL // NCORES  # 8 local batches
NKC = 12  # recurrent contraction chunks: [prev_out (4) | h (8)]
NHC = H // 128  # 8


def bf16(x):
    return np.ascontiguousarray(x.astype(ml_dtypes.bfloat16))


def build_core_kernel(nsc_b, sL_b, T=T_FULL):
    """nsc_b: 128-chunk counts per slot; sL_b: exact max source length per slot."""
    nc = bass.Bass()
    enc_t_cols = [8 * sL for sL in sL_b]  # encT free-cols per batch (exact-s)
    enc_t_off = np.cumsum([0] + enc_t_cols).tolist()
    tot_enc_t = enc_t_off[-1]  # free dim of resident encT
    enc_c_cols = [nsc * 1024 for nsc in nsc_b]  # encC cols per batch
    enc_c_off = np.cumsum([0] + enc_c_cols).tolist()
    tot_enc_c = enc_c_off[-1]

    # ---- DRAM I/O -------------------------------------------------------
    encT_d = nc.dram_tensor("encT", [128, tot_enc_t], BF16, kind="ExternalInput")
    encC_d = nc.dram_tensor("encC", [128, tot_enc_c], BF16, kind="ExternalInput")
    # recurrent weights, quarter-major, 4 kc per 1MB tile: [q, g, 128, 4096]
    wrec_d = nc.dram_tensor("wrec", [4, 3, 128, 4096], BF16, kind="ExternalInput")
    # attn_W as 2 tiles of 4 hc each
    attn_d = nc.dram_tensor("attnW", [2, 128, 4096], BF16, kind="ExternalInput")
    # proj1 as 4 tiles of 4 kc each
    p1_d = nc.dram_tensor("p1T", [4, 128, 4096], BF16, kind="ExternalInput")
    p2_d = nc.dram_tensor("p2T", [NHC, 128, D], BF16, kind="ExternalInput")
    gx_d = nc.dram_tensor("gx", [T, 128, 1024], BF16, kind="ExternalInput")
    h0_d = nc.dram_tensor("h0T", [NHC, 128, B], BF16, kind="ExternalInput")
    o0_d = nc.dram_tensor("o0T", [4, 128, B], BF16, kind="ExternalInput")
    c0_d = nc.dram_tensor("c0", [B, H], F32, kind="ExternalInput")
    valid_d = nc.dram_tensor("valid", [B, S], BF16, kind="ExternalInput")
    rmask_d = nc.dram_tensor("rmask", [40, B * 512], mybir.dt.uint8, kind="ExternalInput")
    out_d = nc.dram_tensor("out", [B, T, D], F32, kind="ExternalOutput")
    P_d = nc.dram_tensor("Pscratch", [128, tot_enc_t], BF16)

    with tile.TileContext(nc) as tc, ExitStack() as ctx:
        const = ctx.enter_context(tc.tile_pool(name="const", bufs=1))
        stream = ctx.enter_context(tc.tile_pool(name="stream", bufs=2))
        gxs = ctx.enter_context(tc.tile_pool(name="gxs", bufs=1))
        work = ctx.enter_context(tc.tile_pool(name="work", bufs=2))
        pgate = ctx.enter_context(tc.tile_pool(name="pgate", bufs=1, space="PSUM"))
        pmid = ctx.enter_context(tc.tile_pool(name="pmid", bufs=1, space="PSUM"))
        ptr = ctx.enter_context(tc.tile_pool(name="ptr", bufs=2, space="PSUM"))
        pjk = ctx.enter_context(tc.tile_pool(name="pjk", bufs=2, space="PSUM"))

        # ---- resident tiles --------------------------------------------
        encT_sb = const.tile([128, tot_enc_t], BF16, name="encT_sb")
        for b_ in range(B):
            nc.sync.dma_start(
                out=encT_sb[:, enc_t_off[b_]:enc_t_off[b_ + 1]],
                in_=encT_d[:, enc_t_off[b_]:enc_t_off[b_ + 1]],
            )
        p2T_sb = const.tile([128, NHC * D], BF16, name="p2T_sb")
        for kc in range(NHC):
            nc.sync.dma_start(out=p2T_sb[:, kc * D:(kc + 1) * D], in_=p2_d[kc])
        idn = const.tile([128, 128], BF16, name="idn")
        make_identity(nc, idn)
        valid_sb = const.tile([B, S], BF16, name="valid_sb")
        nc.sync.dma_start(out=valid_sb[:, :], in_=valid_d[:, :])
        rmask_sb = const.tile([40, B * 512], mybir.dt.uint8, name="rmask_sb")
        nc.sync.dma_start(out=rmask_sb[:, :], in_=rmask_d[:, :])

        # persistent state
        hT = const.tile([128, NHC * B], BF16, name="hT")  # h, k-major
        oT = const.tile([128, 4 * B], BF16, name="oT")  # prev out, k-major
        c_sb = const.tile([B, H], F32, name="c_sb")
        qT = const.tile([128, NHC * B], BF16, name="qT")
        aT = const.tile([128, 8 * B], BF16, name="aT")
        cT = const.tile([128, NHC * B], BF16, name="cT")  # context, k-major
        tyT = const.tile([128, NHC * B], BF16, name="tyT")  # tanh(y), k-major
        scal = const.tile([B, 4], F32, name="scal")  # negmax | den | rden

        for kc in range(NHC):
            nc.sync.dma_start(out=hT[:, kc * B:(kc + 1) * B], in_=h0_d[kc])
        for kc in range(4):
            nc.sync.dma_start(out=oT[:, kc * B:(kc + 1) * B], in_=o0_d[kc])
        nc.sync.dma_start(out=c_sb[:, :], in_=c0_d[:, :])

        AF = mybir.ActivationFunctionType
        OP = mybir.AluOpType

        class StreamMgr:
            def __init__(self):
                self.readers = []  # last-reader inst per allocation

            def tile_dma(self, dram_ap, cols=4096, pool=stream, tag="st", eng=None):
                idx = len(self.readers)
                nb_ = 3 if tag in ("st", "ec") else 2
                tl = pool.tile([128, cols], BF16, tag=tag, name=tag, bufs=nb_)
                (eng or nc.sync).dma_start(out=tl[:, :], in_=dram_ap)
                self.readers.append(None)
                return tl, idx

            def set_reader(self, idx, inst):
                self.readers[idx] = inst

        sm = StreamMgr()

        def transp8(dst_ap, src_ap, base=0):
            """src [B,128] sbuf (partitions base..base+B) -> dst [128,B] sbuf."""
            tp = ptr.tile([128, B], src_ap.dtype, tag="tp", name="tp")
            nc.tensor.transpose(tp[:, :], src_ap, idn[base:base + B, base:base + B])
            nc.vector.tensor_copy(dst_ap, tp[:, :])

        def in_lhsT(kc):
            if kc < 4:
                return oT[:, kc * B:(kc + 1) * B]
            return hT[:, (kc - 4) * B:(kc - 4 + 1) * B]

        def emit_hpart_q(gxt, qi):
            """h @ W_hh quarter qi for the next step's gates, added into gxt."""
            pg2 = pgate.tile([128, H], F32, tag="pg", name="pg2")
            for g in (1, 2):
                wk, wk_i = sm.tile_dma(wrec_d[qi, g])
                last_mm = None
                for j in range(4):
                    kc = g * 4 + j
                    lhsT = hT[:, (kc - 4) * B:(kc - 4 + 1) * B]
                    for nb in range(2):
                        last_mm = nc.tensor.matmul(
                            pg2[32 * qi:32 * qi + B, nb * 512:(nb + 1) * 512],
                            lhsT,
                            wk[:, j * 1024 + nb * 512: j * 1024 + (nb + 1) * 512],
                            start=(kc == 4),
                            stop=(kc == NKC - 1),
                            tile_position=(0, 32 * qi),
                        )
                sm.set_reader(wk_i, last_mm)
            nc.vector.tensor_tensor(
                gxt[32 * qi:32 * qi + B, :],
                pg2[32 * qi:32 * qi + B, :],
                gxt[32 * qi:32 * qi + B, :],
                OP.add,
            )

        # ---- P-phase: overwrite encT with attn_W @ encT (key hoist) ----
        # attn_d now holds attn_W.T packed; scores become h2 . P directly.
        at0, at0_i = sm.tile_dma(attn_d[0])
        at1, at1_i = sm.tile_dma(attn_d[1])
        at_last = [None, None]
        for b in range(B):
            ncols = sL_b[b]
            nblk = (ncols + 511) // 512
            for blk in range(nblk):
                n0 = blk * 512
                n1 = min(ncols, n0 + 512)
                for mc in range(8):
                    pp = pjk.tile([128, 512], F32, tag="pj", name="pp")
                    for ag in range(2):
                        atile = at0 if ag == 0 else at1
                        for j in range(4):
                            kc = ag * 4 + j
                            mm = nc.tensor.matmul(
                                pp[:, 0:n1 - n0],
                                atile[:, j * 1024 + mc * 128: j * 1024 + (mc + 1) * 128],
                                encT_sb[:, enc_t_off[b] + kc * ncols + n0:
                                        enc_t_off[b] + kc * ncols + n1],
                                start=(kc == 0),
                                stop=(kc == 7),
                            )
                            at_last[ag] = mm
                    ps = work.tile([128, 512], BF16, tag="bfw", name="ps", bufs=2)
                    nc.vector.tensor_copy(ps[:, 0:n1 - n0], pp[:, 0:n1 - n0])
                    nc.sync.dma_start(
                        out=P_d[:, enc_t_off[b] + mc * ncols + n0:
                                enc_t_off[b] + mc * ncols + n1],
                        in_=ps[:, 0:n1 - n0],
                    )
        sm.set_reader(at0_i, at_last[0])
        sm.set_reader(at1_i, at_last[1])
        nc.sync.dma_start(out=encT_sb[:, :], in_=P_d[:, :])

        # prologue: gx_0 + h0-part
        gxt = gxs.tile([128, 1024], BF16, tag="gx", name="gxt")
        nc.sync.dma_start(out=gxt[:, :], in_=gx_d[0])
        for qi in range(4):
            emit_hpart_q(gxt, qi)

        for t in range(T):
            gx_last = [None]

            # ---- gates: four quarters i, f, g, o -----------------------
            ptw = {}
            for qi in range(4):
                pg = pgate.tile([B, H], F32, tag="pg", name="pg")
                # seed with gx_t + (h-part accumulated into gxt last iteration)
                for nb in range(2):
                    mm = nc.tensor.matmul(
                        pg[:, nb * 512:(nb + 1) * 512],
                        idn[32 * qi:32 * qi + B, 32 * qi:32 * qi + B],
                        gxt[32 * qi:32 * qi + B, nb * 512:(nb + 1) * 512],
                        start=True,
                        stop=False,
                        tile_position=(32 * qi, 0),
                    )
                    gx_last[0] = mm
                # o-part only (kc 0..3 = wrec group 0)
                wk, wk_i = sm.tile_dma(wrec_d[qi, 0])
                last_mm = None
                for j in range(4):
                    kc = j
                    lhsT = in_lhsT(kc)
                    for nb in range(2):
                        last_mm = nc.tensor.matmul(
                            pg[:, nb * 512:(nb + 1) * 512],
                            lhsT,
                            wk[:, j * 1024 + nb * 512: j * 1024 + (nb + 1) * 512],
                            start=False,
                            stop=(kc == 3),
                        )
                sm.set_reader(wk_i, last_mm)
                gname = ("si", "sf", "tg", "so")[qi]
                g_sb = work.tile([B, H], BF16, tag="pw", name=gname, bufs=4)
                fn = AF.Tanh if gname == "tg" else AF.Sigmoid
                nc.scalar.activation(g_sb[:, :], pg[:, :], fn)
                ptw[gname] = g_sb

            # ---- c/h update -------------------------------------------
            nc.vector.tensor_tensor(c_sb[:, :], ptw["sf"][:, :], c_sb[:, :], OP.mult)
            t2 = work.tile([B, H], BF16, tag="pw", name="t2", bufs=4)
            nc.vector.tensor_tensor(t2[:, :], ptw["si"][:, :], ptw["tg"][:, :], OP.mult)
            nc.vector.tensor_tensor(c_sb[:, :], c_sb[:, :], t2[:, :], OP.add)
            tc2 = work.tile([B, H], BF16, tag="pw", name="tc2", bufs=4)
            nc.scalar.activation(tc2[:, :], c_sb[:, :], AF.Tanh)
            h2 = work.tile([B, H], BF16, tag="bfw", name="h2", bufs=2)
            nc.vector.tensor_tensor(h2[:, :], ptw["so"][:, :], tc2[:, :], OP.mult)
            for hc in range(NHC):
                transp8(hT[:, hc * B:(hc + 1) * B], h2[:, hc * 128:(hc + 1) * 128])

            # ---- scores = q . encT (resident, junk-row trick) ---------
            s_f32 = work.tile([B, S], F32, tag="sf32", name="s_f32", bufs=1)
            nc.vector.memset(s_f32[:, :], 0.0)
            for b in range(B):
                ncols = sL_b[b]
                nhalf = (ncols + 511) // 512
                for nb in range(nhalf):
                    n0 = nb * 512
                    n1 = min(ncols, n0 + 512)
                    pj = pjk.tile([B, 512], F32, tag="pj", name="pj")
                    for hc in range(NHC):
                        base = enc_t_off[b] + hc * ncols
                        nc.tensor.matmul(
                            pj[:, 0:n1 - n0],
                            hT[:, hc * B:(hc + 1) * B],
                            encT_sb[:, base + n0:base + n1],
                            start=(hc == 0),
                            stop=(hc == NHC - 1),
                        )
                    nc.vector.copy_predicated(
                        s_f32[:, n0:n1],
                        rmask_sb[0:B, b * 512:b * 512 + (n1 - n0)],
                        pj[:, 0:n1 - n0],
                    )

            # ---- softmax (masked) -------------------------------------
            nc.vector.tensor_reduce(
                scal[:, 0:1], s_f32[:, :], mybir.AxisListType.X, OP.max, negate=True
            )
            a_bf = work.tile([B, S], BF16, tag="bfa", name="a_bf", bufs=1)
            nc.scalar.activation(a_bf[:, :], s_f32[:, :], AF.Exp, bias=scal[:, 0:1])
            nc.vector.tensor_tensor(a_bf[:, :], a_bf[:, :], valid_sb[:, :], OP.mult)
            nc.vector.tensor_reduce(
                scal[:, 1:2], a_bf[:, :], mybir.AxisListType.X, OP.add
            )
            nc.vector.reciprocal(scal[:, 2:3], scal[:, 1:2])
            nc.vector.tensor_scalar_mul(a_bf[:, :], a_bf[:, :], scal[:, 2:3])
            for sc in range(8):
                transp8(aT[:, sc * B:(sc + 1) * B], a_bf[:, sc * 128:(sc + 1) * 128])

            # ---- next step's gx (h-part quarters interleave with ctx) ----
            if t + 1 < T:
                gxt = gxs.tile([128, 1024], BF16, tag="gx", name="gxt")
                nc.sync.dma_start(out=gxt[:, :], in_=gx_d[t + 1])

            # ---- context = a . enc (streamed, junk-row trick) ---------
            cf = work.tile([B, H], BF16, tag="bfw", name="cf", bufs=2)
            for b in range(B):
                if b % 2 == 1 and t + 1 < T:
                    emit_hpart_q(gxt, b // 2)
                nsc = nsc_b[b]
                ncols = nsc * 1024
                ec0, ec0_i = sm.tile_dma(
                    encC_d[:, enc_c_off[b]:enc_c_off[b] + min(ncols, 4096)],
                    cols=min(ncols, 4096), tag="ec",
                )
                ec1 = ec1_i = None
                if ncols > 4096:
                    ec1, ec1_i = sm.tile_dma(
                        encC_d[:, enc_c_off[b] + 4096:enc_c_off[b] + ncols],
                        cols=ncols - 4096, tag="ec2",
                    )
                pjc = [pjk.tile([B, 512], F32, tag="pj", name="pjc") for _ in range(2)]
                last0 = last1 = None
                for sc in range(nsc):
                    tl = ec0 if sc < 4 else ec1
                    off = (sc % 4) * 1024
                    for nb in range(2):
                        mm = nc.tensor.matmul(
                            pjc[nb][:, :],
                            aT[:, sc * B:(sc + 1) * B],
                            tl[:, off + nb * 512: off + (nb + 1) * 512],
                            start=(sc == 0),
                            stop=(sc == nsc - 1),
                        )
                        if sc < 4:
                            last0 = mm
                        else:
                            last1 = mm
                sm.set_reader(ec0_i, last0)
                if ec1 is not None:
                    sm.set_reader(ec1_i, last1 if last1 is not None else last0)
                for nb in range(2):
                    nc.vector.copy_predicated(
                        cf[:, nb * 512:(nb + 1) * 512],
                        rmask_sb[0:B, b * 512:(b + 1) * 512],
                        pjc[nb][:, :],
                    )
            # ---- y = [h2, ctx] @ proj1.T: h-half early ----------------
            py = pmid.tile([B, H], F32, tag="pm", name="py")
            for pg_i in range(2):
                p1, p1_i = sm.tile_dma(p1_d[pg_i])
                last_mm = None
                for j in range(4):
                    kc = pg_i * 4 + j
                    lhsT = hT[:, kc * B:(kc + 1) * B]
                    for nb in range(2):
                        last_mm = nc.tensor.matmul(
                            py[:, nb * 512:(nb + 1) * 512],
                            lhsT,
                            p1[:, j * 1024 + nb * 512: j * 1024 + (nb + 1) * 512],
                            start=(kc == 0),
                            stop=False,
                        )
                sm.set_reader(p1_i, last_mm)
            for kc in range(NHC):
                transp8(cT[:, kc * B:(kc + 1) * B], cf[:, kc * 128:(kc + 1) * 128])
            for pg_i in (2, 3):
                p1, p1_i = sm.tile_dma(p1_d[pg_i])
                last_mm = None
                for j in range(4):
                    kc = pg_i * 4 + j
                    lhsT = cT[:, (kc - 8) * B:(kc - 8 + 1) * B]
                    for nb in range(2):
                        last_mm = nc.tensor.matmul(
                            py[:, nb * 512:(nb + 1) * 512],
                            lhsT,
                            p1[:, j * 1024 + nb * 512: j * 1024 + (nb + 1) * 512],
                            start=False,
                            stop=(kc == 15),
                        )
                sm.set_reader(p1_i, last_mm)
            ty = work.tile([B, H], BF16, tag="bfw", name="ty", bufs=2)
            nc.scalar.activation(ty[:, :], py[:, :], AF.Tanh)
            for kc in range(NHC):
                transp8(tyT[:, kc * B:(kc + 1) * B], ty[:, kc * 128:(kc + 1) * 128])

            # ---- out = ty @ proj2.T -----------------------------------
            po = pmid.tile([B, D], F32, tag="pm", name="po")
            for kc in range(NHC):
                nc.tensor.matmul(
                    po[:, :],
                    tyT[:, kc * B:(kc + 1) * B],
                    p2T_sb[:, kc * D:(kc + 1) * D],
                    start=(kc == 0),
                    stop=(kc == NHC - 1),
                )
            of = work.tile([B, D], F32, tag="ofw", name="of", bufs=1)
            of_cp = nc.scalar.activation(of[:, :], po[:, :], AF.Copy)
            ob = work.tile([B, D], BF16, tag="bfw", name="ob", bufs=2)
            nc.vector.tensor_copy(ob[:, :], po[:, :])
            nc.sync.dma_start(out=out_d[:, t, :], in_=of[:, :])
            for kc in range(4):
                transp8(oT[:, kc * B:(kc + 1) * B], ob[:, kc * 128:(kc + 1) * 128])

    return nc


def _sorted_assignment(slen_all):
    """rank r -> (core r%8, slot r//8); returns per-core batch lists and
    per-slot chunk counts (shared across cores for one NEFF)."""
    order = np.argsort(np.asarray(slen_all), kind="stable")
    batch_lists = [[int(order[s * NCORES + c]) for s in range(B)] for c in range(NCORES)]
    sL_b = [
        int(max(slen_all[order[s * NCORES + c]] for c in range(NCORES)))
        for s in range(B)
    ]
    nsc_b = [int(np.ceil(sL / 128.0)) for sL in sL_b]
    return batch_lists, nsc_b, sL_b


def _prep_core_inputs(inputs, bl, nsc_b, sL_b, gx_full, T=T_FULL):
    """bl: list of 8 global batch indices for this core (slot order)."""
    bl = list(bl)
    enc = np.asarray(inputs["enc_outs"][bl], np.float32)  # [B,S,H]
    h0 = np.asarray(inputs["init_h"][-1][bl], np.float32)  # [B,H]
    c0 = np.asarray(inputs["init_c"][-1][bl], np.float32)
    mask = np.asarray(inputs["source_rep_mask"][bl])  # [B,S] bool
    slen = np.asarray(inputs["source_length"][bl]).astype(np.float32)
    p1W = np.asarray(inputs["proj1_W"], np.float32)
    p1b = np.asarray(inputs["proj1_b"], np.float32)
    p2W = np.asarray(inputs["proj2_W"], np.float32)

    valid = (~mask).astype(np.float32)
    # init_out on host (exact fp32)
    seq_mean = (enc * valid[:, :, None]).sum(1) / slen[:, None]
    cat = np.concatenate([h0, seq_mean], -1)
    init_out = np.tanh(cat @ p1W.T + p1b) @ p2W.T  # [B,D]

    # encT resident: per batch, [hc, 128, ncols] trimmed+padded
    tot = sum(8 * sL for sL in sL_b)
    encT = np.zeros((128, tot), np.float32)
    off = 0
    for b in range(B):
        ncols = sL_b[b]
        e = np.zeros((H, ncols), np.float32)
        sv = min(S, ncols)
        e[:, :sv] = enc[b, :sv, :].T
        e = e.reshape(8, 128, ncols)
        for hc in range(8):
            encT[:, off:off + ncols] = e[hc]
            off += ncols

    # encC packed: [128, sum_b nsc_b*1024]; col sc*1024+h, partition = s%128
    tot_c = sum(nsc * 1024 for nsc in nsc_b)
    encC = np.zeros((128, tot_c), np.float32)
    off = 0
    for b in range(B):
        for sc in range(nsc_b[b]):
            encC[:, off:off + 1024] = enc[b, sc * 128:(sc + 1) * 128, :]
            off += 1024
    del enc

    # gx packed [T, 128, 1024]: quarter q of batch b at partition 32q+b
    gxb = gx_full[bl].transpose(1, 0, 2)  # [T, B, 4096]
    Tn = gxb.shape[0]
    arr = gxb.reshape(Tn, B, 4, 1024).transpose(0, 2, 1, 3)  # [T, 4, B, 1024]
    gx = np.zeros((Tn, 4, 32, 1024), np.float32)
    gx[:, :, :B, :] = arr
    gx = gx.reshape(Tn, 128, 1024)

    rmask = np.zeros((B, B, 512), np.float32)
    for b in range(B):
        rmask[b, b, :] = 1.0
    rmask = rmask.transpose(1, 0, 2).reshape(B, B * 512)
    rmask40 = np.zeros((40, B * 512), np.float32)
    rmask40[0:B] = rmask
    rmask40[32:32 + B] = rmask
    return {
        "rmask": rmask40.astype(np.uint8),
        "encT": bf16(encT),
        "encC": bf16(encC),
        "gx": bf16(gx),
        "h0T": bf16(h0.T.reshape(NHC, 128, B)),
        "o0T": bf16(init_out.T.reshape(4, 128, B)),
        "c0": np.ascontiguousarray(c0),
        "valid": bf16(valid),
    }


def _prep_shared_weights(inputs):
    W_ih = np.asarray(inputs["W_ih"], np.float32)
    W_hh = np.asarray(inputs["W_hh"], np.float32)
    attn_W = np.asarray(inputs["attn_W"], np.float32)
    p1W = np.asarray(inputs["proj1_W"], np.float32)
    p2W = np.asarray(inputs["proj2_W"], np.float32)

    # recurrent weights [prev_out | h]: Wcat [4096, 1536] -> blocks kc [128,4096]
    Wcat = np.concatenate([W_ih[:, D:], W_hh], axis=1)  # [4H, 1536]
    blocks = Wcat.T.reshape(NKC, 128, 4096)  # kc-major
    wrec = np.zeros((4, 3, 128, 4096), np.float32)
    for q in range(4):
        for g in range(3):
            for j in range(4):
                wrec[q, g, :, j * 1024:(j + 1) * 1024] = (
                    blocks[g * 4 + j][:, q * 1024:(q + 1) * 1024]
                )
    # attn [2, 128, 4096]: tile ag, col j*1024+n = attn block (hc=ag*4+j)
    ablocks = np.ascontiguousarray(attn_W.T).reshape(NHC, 128, H)
    attn = np.zeros((2, 128, 4096), np.float32)
    for ag in range(2):
        for j in range(4):
            attn[ag, :, j * 1024:(j + 1) * 1024] = ablocks[ag * 4 + j]
    # p1 [4, 128, 4096]
    pblocks = p1W.T.reshape(16, 128, H)
    p1 = np.zeros((4, 128, 4096), np.float32)
    for pg in range(4):
        for j in range(4):
            p1[pg, :, j * 1024:(j + 1) * 1024] = pblocks[pg * 4 + j]
    return {
        "wrec": bf16(wrec),
        "attnW": bf16(attn),
        "p1T": bf16(p1),
        "p2T": bf16(p2W.T.reshape(NHC, 128, D)),
    }


def run(inputs, T=T_FULL, trace=False):
    slen_all = np.asarray(inputs["source_length"]).astype(np.int64)
    batch_lists, nsc_b, sL_b = _sorted_assignment(slen_all)
    nc = build_core_kernel(nsc_b, sL_b, T=T)

    # hoisted x-part of the gates for all (b, t): [64, 64, 4096] fp32
    target = np.asarray(inputs["target"], np.float32)
    Wx = np.asarray(inputs["W_ih"], np.float32)[:, :D]  # [4096, 512]
    gx_full = np.ascontiguousarray(
        np.tensordot(target[:, :T], Wx, axes=([2], [1]))
    )  # [64, T, 4096]

    shared = _prep_shared_weights(inputs)
    in_maps = []
    for c in range(NCORES):
        m = _prep_core_inputs(inputs, batch_lists[c], nsc_b, sL_b, gx_full, T=T)
        m.update(shared)
        in_maps.append(m)
    res = run_bass_kernel_spmd(nc, in_maps, core_ids=list(range(NCORES)), trace=trace)
    out = np.zeros((B_FULL, T, D), np.float32)
    for c in range(NCORES):
        o = res.results[c]["out"]
        for s in range(B):
            out[batch_lists[c][s]] = o[s]
    return out, res


def kernel(**inputs) -> np.ndarray:
    out, _ = run(inputs)
    return out


if __name__ == "__main__":
    np.random.seed(0)
    print("smoke build only")
    nc = build_core_kernel([5, 6, 6, 6, 7, 7, 8, 8], [611, 653, 706, 761, 822, 888, 955, 1018], T=2)
    print("build ok")


# revision 29
# speedup vs baseline: 1.1046x; 1.1046x over previous
"""AttnLSTMDecoder Trainium2 kernel (v2).

Data-parallel over batch: 8 NeuronCores x 8 batches each. The T=64
recurrence runs locally per core; no collectives.

v2 changes vs v1:
  - x-hoist: x_t @ W_x^T precomputed on host for all t (GX); the
    per-step gates matmul only contracts over [prev_out | h] (12 kc
    instead of 16), seeded into PSUM via an identity matmul on GX.
  - Sorted batch assignment: batches sorted by source_length, rank r
    -> (core r%8, slot r//8), so the shared-NEFF per-slot max s-chunk
    counts shrink (63 -> 53 chunks): less SBUF, less encC streaming,
    fewer score/context columns.
  - Coalesced DMA: weights stream as [128, 4096] 1MB tiles (4 kc per
    tile), encC as 2 DMAs per batch from a packed DRAM image.
"""

import os
import sys
from contextlib import ExitStack

import numpy as np

sys.path.insert(0, "/opt/trn_rl_repo")

import ml_dtypes  # noqa: E402

import concourse.bass as bass  # noqa: E402
import concourse.mybir as mybir  # noqa: E402
import concourse.tile as tile  # noqa: E402
import json as _json  # noqa: E402

import concourse.bass_utils as _bu  # noqa: E402
import concourse.bass2jax as _b2j  # noqa: E402
from concourse.bass_utils import run_bass_kernel_spmd  # noqa: E402

_orig_compile_bir_kernel = _bu.compile_bir_kernel


def _strip_ring_waits(bir_str):
    """Work around walrus per-instruction sem-wait limits.

    - DMACopy allows 1 wait: drop DMA ring-throttle waits (redundant with the
      slot-release engine wait; HW rings are deep vs our <=10 in-flight).
    - Engine instrs allow 2: hoist excess waits into a standalone
      EventSemaphore on the same engine directly before the instruction.
    """
    bir = _json.loads(bir_str)
    counter = [0]

    def fix_dma(inst):
        si = inst.get("sync_info")
        if not si:
            return
        ow = si.get("on_wait") or []
        if len(ow) <= 1:
            return
        eng = [w for w in ow if not w.get("ant_name", "").startswith(("DMAHW", "DMASW"))]
        si["on_wait"] = eng[:1] if eng else ow[:1]

    def walk(o):
        if isinstance(o, dict):
            for k, v in o.items():
                if (
                    isinstance(v, list)
                    and v
                    and isinstance(v[0], dict)
                    and "opcode" in v[0]
                ):
                    new = []
                    for inst in v:
                        if inst.get("opcode") == "DMACopy":
                            fix_dma(inst)
                        else:
                            si = inst.get("sync_info") or {}
                            ow = si.get("on_wait") or []
                            if len(ow) > 1:
                                for w in ow[:-1]:
                                    counter[0] += 1
                                    new.append({
                                        "debug": inst.get("debug", 0),
                                        "engine": inst["engine"],
                                        "ins": [],
                                        "name": f"hoist_wait_{counter[0]}",
                                        "opcode": "EventSemaphore",
                                        "outs": [],
                                        "sync_info": {
                                            "on_update": [],
                                            "on_wait": [w],
                                        },
                                    })
                                si["on_wait"] = ow[-1:]
                        new.append(inst)
                        walk(inst)
                    o[k] = new
                elif isinstance(v, (dict, list)):
                    walk(v)
        elif isinstance(o, list):
            for v in o:
                walk(v)

    walk(bir)
    return _json.dumps(bir)


def _patched_compile_bir_kernel(ant_bir_str, *a, **k):
    if isinstance(ant_bir_str, bytes):
        fixed = _strip_ring_waits(ant_bir_str.decode()).encode()
    else:
        fixed = _strip_ring_waits(ant_bir_str)
    return _orig_compile_bir_kernel(fixed, *a, **k)


_bu.compile_bir_kernel = _patched_compile_bir_kernel
_b2j.compile_bir_kernel = _patched_compile_bir_kernel
from concourse.masks import make_identity  # noqa: E402

BF16 = mybir.dt.bfloat16
F32 = mybir.dt.float32

B_FULL, S, T_FULL = 64, 1024, 64
H = 1024
D = 512
NCORES = 8
B = B_FULL // NCORES  # 8 local batches
NKC = 12  # recurrent contraction chunks: [prev_out (4) | h (8)]
NHC = H // 128  # 8


def bf16(x):
    return np.ascontiguousarray(x.astype(ml_dtypes.bfloat16))


def build_core_kernel(nsc_b, sL_b, T=T_FULL):
    """nsc_b: 128-chunk counts per slot; sL_b: exact max source length per slot."""
    nc = bass.Bass()
    enc_t_cols = [8 * sL for sL in sL_b]  # encT free-cols per batch (exact-s)
    enc_t_off = np.cumsum([0] + enc_t_cols).tolist()
    tot_enc_t = enc_t_off[-1]  # free dim of resident encT
    enc_c_cols = [nsc * 1024 for nsc in nsc_b]  # encC cols per batch
    enc_c_off = np.cumsum([0] + enc_c_cols).tolist()
    tot_enc_c = enc_c_off[-1]

    # ---- DRAM I/O -------------------------------------------------------
    encT_d = nc.dram_tensor("encT", [128, tot_enc_t], BF16, kind="ExternalInput")
    encC_d = nc.dram_tensor("encC", [128, tot_enc_c], BF16, kind="ExternalInput")
    # recurrent weights, quarter-major, 4 kc per 1MB tile: [q, g, 128, 4096]
    wrec_d = nc.dram_tensor("wrec", [4, 3, 128, 4096], BF16, kind="ExternalInput")
    # attn_W as 2 tiles of 4 hc each
    attn_d = nc.dram_tensor("attnW", [2, 128, 4096], BF16, kind="ExternalInput")
    # proj1 as 4 tiles of 4 kc each
    p1_d = nc.dram_tensor("p1T", [4, 128, 4096], BF16, kind="ExternalInput")
    p2_d = nc.dram_tensor("p2T", [NHC, 128, D], BF16, kind="ExternalInput")
    gx_d = nc.dram_tensor("gx", [T, B, 4096], BF16, kind="ExternalInput")
    h0_d = nc.dram_tensor("h0T", [NHC, 128, B], BF16, kind="ExternalInput")
    o0_d = nc.dram_tensor("o0T", [4, 128, B], BF16, kind="ExternalInput")
    c0_d = nc.dram_tensor("c0", [B, H], F32, kind="ExternalInput")
    valid_d = nc.dram_tensor("valid", [B, S], BF16, kind="ExternalInput")
    rmask_d = nc.dram_tensor("rmask", [40, B * 512], mybir.dt.uint8, kind="ExternalInput")
    out_d = nc.dram_tensor("out", [B, T, D], F32, kind="ExternalOutput")
    P_d = nc.dram_tensor("Pscratch", [128, tot_enc_t], BF16)

    with tile.TileContext(nc) as tc, ExitStack() as ctx:
        const = ctx.enter_context(tc.tile_pool(name="const", bufs=1))
        stream = ctx.enter_context(tc.tile_pool(name="stream", bufs=2))
        gxs = ctx.enter_context(tc.tile_pool(name="gxs", bufs=1))
        work = ctx.enter_context(tc.tile_pool(name="work", bufs=2))
        pgate = ctx.enter_context(tc.tile_pool(name="pgate", bufs=1, space="PSUM"))
        pmid = ctx.enter_context(tc.tile_pool(name="pmid", bufs=1, space="PSUM"))
        ptr = ctx.enter_context(tc.tile_pool(name="ptr", bufs=2, space="PSUM"))
        pjk = ctx.enter_context(tc.tile_pool(name="pjk", bufs=2, space="PSUM"))

        # ---- resident tiles --------------------------------------------
        encT_sb = const.tile([128, tot_enc_t], BF16, name="encT_sb")
        for b_ in range(B):
            nc.sync.dma_start(
                out=encT_sb[:, enc_t_off[b_]:enc_t_off[b_ + 1]],
                in_=encT_d[:, enc_t_off[b_]:enc_t_off[b_ + 1]],
            )
        p2T_sb = const.tile([128, NHC * D], BF16, name="p2T_sb")
        for kc in range(NHC):
            nc.sync.dma_start(out=p2T_sb[:, kc * D:(kc + 1) * D], in_=p2_d[kc])
        idn = const.tile([128, 128], BF16, name="idn")
        make_identity(nc, idn)
        valid_sb = const.tile([B, S], BF16, name="valid_sb")
        nc.sync.dma_start(out=valid_sb[:, :], in_=valid_d[:, :])
        rmask_sb = const.tile([40, B * 512], mybir.dt.uint8, name="rmask_sb")
        nc.sync.dma_start(out=rmask_sb[:, :], in_=rmask_d[:, :])

        # persistent state
        hT = const.tile([128, NHC * B], BF16, name="hT")  # h, k-major
        oT = const.tile([128, 4 * B], BF16, name="oT")  # prev out, k-major
        c_sb = const.tile([B, H], F32, name="c_sb")
        qT = const.tile([128, NHC * B], BF16, name="qT")
        aT = const.tile([128, 8 * B], BF16, name="aT")
        cT = const.tile([128, NHC * B], BF16, name="cT")  # context, k-major
        tyT = const.tile([128, NHC * B], BF16, name="tyT")  # tanh(y), k-major
        scal = const.tile([B, 4], F32, name="scal")  # negmax | den | rden

        for kc in range(NHC):
            nc.sync.dma_start(out=hT[:, kc * B:(kc + 1) * B], in_=h0_d[kc])
        for kc in range(4):
            nc.sync.dma_start(out=oT[:, kc * B:(kc + 1) * B], in_=o0_d[kc])
        nc.sync.dma_start(out=c_sb[:, :], in_=c0_d[:, :])

        AF = mybir.ActivationFunctionType
        OP = mybir.AluOpType

        class StreamMgr:
            def __init__(self):
                self.readers = []  # last-reader inst per allocation

            def tile_dma(self, dram_ap, cols=4096, pool=stream, tag="st", eng=None):
                idx = len(self.readers)
                nb_ = 3 if tag == "st" else 2
                tl = pool.tile([128, cols], BF16, tag=tag, name=tag, bufs=nb_)
                (eng or nc.sync).dma_start(out=tl[:, :], in_=dram_ap)
                self.readers.append(None)
                return tl, idx

            def set_reader(self, idx, inst):
                self.readers[idx] = inst

        sm = StreamMgr()

        def transp8(dst_ap, src_ap, base=0):
            """src [B,128] sbuf (partitions base..base+B) -> dst [128,B] sbuf."""
            tp = ptr.tile([128, B], src_ap.dtype, tag="tp", name="tp")
            nc.tensor.transpose(tp[:, :], src_ap, idn[base:base + B, base:base + B])
            nc.vector.tensor_copy(dst_ap, tp[:, :])

        def in_lhsT(kc):
            if kc < 4:
                return oT[:, kc * B:(kc + 1) * B]
            return hT[:, (kc - 4) * B:(kc - 4 + 1) * B]

        def emit_hpart_q(gxt, qi):
            """h @ W_hh quarter qi for the next step's gates, added into gxt."""
            pg2 = pgate.tile([B, H], F32, tag="pg", name="pg2")
            for g in (1, 2):
                wk, wk_i = sm.tile_dma(wrec_d[qi, g])
                last_mm = None
                for j in range(4):
                    kc = g * 4 + j
                    lhsT = hT[:, (kc - 4) * B:(kc - 4 + 1) * B]
                    for nb in range(2):
                        last_mm = nc.tensor.matmul(
                            pg2[:, nb * 512:(nb + 1) * 512],
                            lhsT,
                            wk[:, j * 1024 + nb * 512: j * 1024 + (nb + 1) * 512],
                            start=(kc == 4),
                            stop=(kc == NKC - 1),
                        )
                sm.set_reader(wk_i, last_mm)
            nc.vector.tensor_tensor(
                gxt[:, qi * 1024:(qi + 1) * 1024],
                pg2[:, :],
                gxt[:, qi * 1024:(qi + 1) * 1024],
                OP.add,
            )

        # ---- P-phase: overwrite encT with attn_W @ encT (key hoist) ----
        # attn_d now holds attn_W.T packed; scores become h2 . P directly.
        at0, at0_i = sm.tile_dma(attn_d[0])
        at1, at1_i = sm.tile_dma(attn_d[1])
        at_last = [None, None]
        for b in range(B):
            ncols = sL_b[b]
            nblk = (ncols + 511) // 512
            for blk in range(nblk):
                n0 = blk * 512
                n1 = min(ncols, n0 + 512)
                for mc in range(8):
                    pp = pjk.tile([128, 512], F32, tag="pj", name="pp")
                    for ag in range(2):
                        atile = at0 if ag == 0 else at1
                        for j in range(4):
                            kc = ag * 4 + j
                            mm = nc.tensor.matmul(
                                pp[:, 0:n1 - n0],
                                atile[:, j * 1024 + mc * 128: j * 1024 + (mc + 1) * 128],
                                encT_sb[:, enc_t_off[b] + kc * ncols + n0:
                                        enc_t_off[b] + kc * ncols + n1],
                                start=(kc == 0),
                                stop=(kc == 7),
                            )
                            at_last[ag] = mm
                    ps = work.tile([128, 512], BF16, tag="bfw", name="ps", bufs=3)
                    nc.vector.tensor_copy(ps[:, 0:n1 - n0], pp[:, 0:n1 - n0])
                    nc.sync.dma_start(
                        out=P_d[:, enc_t_off[b] + mc * ncols + n0:
                                enc_t_off[b] + mc * ncols + n1],
                        in_=ps[:, 0:n1 - n0],
                    )
        sm.set_reader(at0_i, at_last[0])
        sm.set_reader(at1_i, at_last[1])
        nc.sync.dma_start(out=encT_sb[:, :], in_=P_d[:, :])

        # prologue: gx_0 + h0-part
        gxt = gxs.tile([B, 4096], BF16, tag="gx", name="gxt")
        nc.sync.dma_start(out=gxt[:, :], in_=gx_d[0])
        for qi in range(4):
            emit_hpart_q(gxt, qi)

        for t in range(T):
            gx_last = [None]

            # ---- gates: four quarters i, f, g, o -----------------------
            ptw = {}
            for qi in range(4):
                pg = pgate.tile([B, H], F32, tag="pg", name="pg")
                # seed with gx_t + (h-part accumulated into gxt last iteration)
                for nb in range(2):
                    mm = nc.tensor.matmul(
                        pg[:, nb * 512:(nb + 1) * 512],
                        idn[:B, :B],
                        gxt[:, qi * 1024 + nb * 512: qi * 1024 + (nb + 1) * 512],
                        start=True,
                        stop=False,
                    )
                    gx_last[0] = mm
                # o-part only (kc 0..3 = wrec group 0)
                wk, wk_i = sm.tile_dma(wrec_d[qi, 0])
                last_mm = None
                for j in range(4):
                    kc = j
                    lhsT = in_lhsT(kc)
                    for nb in range(2):
                        last_mm = nc.tensor.matmul(
                            pg[:, nb * 512:(nb + 1) * 512],
                            lhsT,
                            wk[:, j * 1024 + nb * 512: j * 1024 + (nb + 1) * 512],
                            start=False,
                            stop=(kc == 3),
                        )
                sm.set_reader(wk_i, last_mm)
                gname = ("si", "sf", "tg", "so")[qi]
                g_sb = work.tile([B, H], BF16, tag="pw", name=gname, bufs=4)
                fn = AF.Tanh if gname == "tg" else AF.Sigmoid
                nc.scalar.activation(g_sb[:, :], pg[:, :], fn)
                ptw[gname] = g_sb

            # ---- c/h update -------------------------------------------
            nc.vector.tensor_tensor(c_sb[:, :], ptw["sf"][:, :], c_sb[:, :], OP.mult)
            t2 = work.tile([B, H], BF16, tag="pw", name="t2", bufs=4)
            nc.vector.tensor_tensor(t2[:, :], ptw["si"][:, :], ptw["tg"][:, :], OP.mult)
            nc.vector.tensor_tensor(c_sb[:, :], c_sb[:, :], t2[:, :], OP.add)
            tc2 = work.tile([B, H], BF16, tag="pw", name="tc2", bufs=4)
            nc.scalar.activation(tc2[:, :], c_sb[:, :], AF.Tanh)
            h2 = work.tile([B, H], BF16, tag="bfw", name="h2", bufs=3)
            nc.vector.tensor_tensor(h2[:, :], ptw["so"][:, :], tc2[:, :], OP.mult)
            for hc in range(NHC):
                transp8(hT[:, hc * B:(hc + 1) * B], h2[:, hc * 128:(hc + 1) * 128])

            # ---- scores = q . encT (resident, junk-row trick) ---------
            s_f32 = work.tile([B, S], F32, tag="sf32", name="s_f32", bufs=1)
            nc.vector.memset(s_f32[:, :], 0.0)
            for b in range(B):
                ncols = sL_b[b]
                nhalf = (ncols + 511) // 512
                for nb in range(nhalf):
                    n0 = nb * 512
                    n1 = min(ncols, n0 + 512)
                    pj = pjk.tile([B, 512], F32, tag="pj", name="pj")
                    for hc in range(NHC):
                        base = enc_t_off[b] + hc * ncols
                        nc.tensor.matmul(
                            pj[:, 0:n1 - n0],
                            hT[:, hc * B:(hc + 1) * B],
                            encT_sb[:, base + n0:base + n1],
                            start=(hc == 0),
                            stop=(hc == NHC - 1),
                        )
                    nc.vector.copy_predicated(
                        s_f32[:, n0:n1],
                        rmask_sb[0:B, b * 512:b * 512 + (n1 - n0)],
                        pj[:, 0:n1 - n0],
                    )

            # ---- softmax (masked) -------------------------------------
            nc.vector.tensor_reduce(
                scal[:, 0:1], s_f32[:, :], mybir.AxisListType.X, OP.max, negate=True
            )
            a_bf = work.tile([B, S], BF16, tag="bfa", name="a_bf", bufs=1)
            nc.scalar.activation(a_bf[:, :], s_f32[:, :], AF.Exp, bias=scal[:, 0:1])
            nc.vector.tensor_tensor(a_bf[:, :], a_bf[:, :], valid_sb[:, :], OP.mult)
            nc.vector.tensor_reduce(
                scal[:, 1:2], a_bf[:, :], mybir.AxisListType.X, OP.add
            )
            nc.vector.reciprocal(scal[:, 2:3], scal[:, 1:2])
            nc.vector.tensor_scalar_mul(a_bf[:, :], a_bf[:, :], scal[:, 2:3])
            for sc in range(8):
                transp8(aT[:, sc * B:(sc + 1) * B], a_bf[:, sc * 128:(sc + 1) * 128])

            # ---- next step's gx (h-part quarters interleave with ctx) ----
            if t + 1 < T:
                gxt = gxs.tile([B, 4096], BF16, tag="gx", name="gxt")
                nc.sync.dma_start(out=gxt[:, :], in_=gx_d[t + 1])

            # ---- context = a . enc (streamed, junk-row trick) ---------
            cf = work.tile([B, H], BF16, tag="bfw", name="cf", bufs=3)
            for b in range(B):
                if b % 2 == 1 and t + 1 < T:
                    emit_hpart_q(gxt, b // 2)
                nsc = nsc_b[b]
                ncols = nsc * 1024
                ec0, ec0_i = sm.tile_dma(
                    encC_d[:, enc_c_off[b]:enc_c_off[b] + min(ncols, 4096)],
                    cols=min(ncols, 4096), tag="ec",
                )
                ec1 = ec1_i = None
                if ncols > 4096:
                    ec1, ec1_i = sm.tile_dma(
                        encC_d[:, enc_c_off[b] + 4096:enc_c_off[b] + ncols],
                        cols=ncols - 4096, tag="ec2",
                    )
                pjc = [pjk.tile([B, 512], F32, tag="pj", name="pjc") for _ in range(2)]
                last0 = last1 = None
                for sc in range(nsc):
                    tl = ec0 if sc < 4 else ec1
                    off = (sc % 4) * 1024
                    for nb in range(2):
                        mm = nc.tensor.matmul(
                            pjc[nb][:, :],
                            aT[:, sc * B:(sc + 1) * B],
                            tl[:, off + nb * 512: off + (nb + 1) * 512],
                            start=(sc == 0),
                            stop=(sc == nsc - 1),
                        )
                        if sc < 4:
                            last0 = mm
                        else:
                            last1 = mm
                sm.set_reader(ec0_i, last0)
                if ec1 is not None:
                    sm.set_reader(ec1_i, last1 if last1 is not None else last0)
                for nb in range(2):
                    nc.vector.copy_predicated(
                        cf[:, nb * 512:(nb + 1) * 512],
                        rmask_sb[0:B, b * 512:(b + 1) * 512],
                        pjc[nb][:, :],
                    )
            # ---- y = [h2, ctx] @ proj1.T: h-half early ----------------
            py = pmid.tile([B, H], F32, tag="pm", name="py")
            for pg_i in range(2):
                p1, p1_i = sm.tile_dma(p1_d[pg_i])
                last_mm = None
                for j in range(4):
                    kc = pg_i * 4 + j
                    lhsT = hT[:, kc * B:(kc + 1) * B]
                    for nb in range(2):
                        last_mm = nc.tensor.matmul(
                            py[:, nb * 512:(nb + 1) * 512],
                            lhsT,
                            p1[:, j * 1024 + nb * 512: j * 1024 + (nb + 1) * 512],
                            start=(kc == 0),
                            stop=False,
                        )
                sm.set_reader(p1_i, last_mm)
            for kc in range(NHC):
                transp8(cT[:, kc * B:(kc + 1) * B], cf[:, kc * 128:(kc + 1) * 128])
            for pg_i in (2, 3):
                p1, p1_i = sm.tile_dma(p1_d[pg_i])
                last_mm = None
                for j in range(4):
                    kc = pg_i * 4 + j
                    lhsT = cT[:, (kc - 8) * B:(kc - 8 + 1) * B]
                    for nb in range(2):
                        last_mm = nc.tensor.matmul(
                            py[:, nb * 512:(nb + 1) * 512],
                            lhsT,
                            p1[:, j * 1024 + nb * 512: j * 1024 + (nb + 1) * 512],
                            start=False,
                            stop=(kc == 15),
                        )
                sm.set_reader(p1_i, last_mm)
            ty = work.tile([B, H], BF16, tag="bfw", name="ty", bufs=3)
            nc.scalar.activation(ty[:, :], py[:, :], AF.Tanh)
            for kc in range(NHC):
                transp8(tyT[:, kc * B:(kc + 1) * B], ty[:, kc * 128:(kc + 1) * 128])

            # ---- out = ty @ proj2.T -----------------------------------
            po = pmid.tile([B, D], F32, tag="pm", name="po")
            for kc in range(NHC):
                nc.tensor.matmul(
                    po[:, :],
                    tyT[:, kc * B:(kc + 1) * B],
                    p2T_sb[:, kc * D:(kc + 1) * D],
                    start=(kc == 0),
                    stop=(kc == NHC - 1),
                )
            of = work.tile([B, D], F32, tag="ofw", name="of", bufs=2)
            of_cp = nc.scalar.activation(of[:, :], po[:, :], AF.Copy)
            ob = work.tile([B, D], BF16, tag="bfw", name="ob", bufs=3)
            nc.vector.tensor_copy(ob[:, :], po[:, :])
            nc.sync.dma_start(out=out_d[:, t, :], in_=of[:, :])
            for kc in range(4):
                transp8(oT[:, kc * B:(kc + 1) * B], ob[:, kc * 128:(kc + 1) * 128])

    return nc


def _sorted_assignment(slen_all):
    """rank r -> (core r%8, slot r//8); returns per-core batch lists and
    per-slot chunk counts (shared across cores for one NEFF)."""
    order = np.argsort(np.asarray(slen_all), kind="stable")
    batch_lists = [[int(order[s * NCORES + c]) for s in range(B)] for c in range(NCORES)]
    sL_b = [
        int(max(slen_all[order[s * NCORES + c]] for c in range(NCORES)))
        for s in range(B)
    ]
    nsc_b = [int(np.ceil(sL / 128.0)) for sL in sL_b]
    return batch_lists, nsc_b, sL_b


def _prep_core_inputs(inputs, bl, nsc_b, sL_b, gx_full, T=T_FULL):
    """bl: list of 8 global batch indices for this core (slot order)."""
    bl = list(bl)
    enc = np.asarray(inputs["enc_outs"][bl], np.float32)  # [B,S,H]
    h0 = np.asarray(inputs["init_h"][-1][bl], np.float32)  # [B,H]
    c0 = np.asarray(inputs["init_c"][-1][bl], np.float32)
    mask = np.asarray(inputs["source_rep_mask"][bl])  # [B,S] bool
    slen = np.asarray(inputs["source_length"][bl]).astype(np.float32)
    p1W = np.asarray(inputs["proj1_W"], np.float32)
    p1b = np.asarray(inputs["proj1_b"], np.float32)
    p2W = np.asarray(inputs["proj2_W"], np.float32)

    valid = (~mask).astype(np.float32)
    # init_out on host (exact fp32)
    seq_mean = (enc * valid[:, :, None]).sum(1) / slen[:, None]
    cat = np.concatenate([h0, seq_mean], -1)
    init_out = np.tanh(cat @ p1W.T + p1b) @ p2W.T  # [B,D]

    # encT resident: per batch, [hc, 128, ncols] trimmed+padded
    tot = sum(8 * sL for sL in sL_b)
    encT = np.zeros((128, tot), np.float32)
    off = 0
    for b in range(B):
        ncols = sL_b[b]
        e = np.zeros((H, ncols), np.float32)
        sv = min(S, ncols)
        e[:, :sv] = enc[b, :sv, :].T
        e = e.reshape(8, 128, ncols)
        for hc in range(8):
            encT[:, off:off + ncols] = e[hc]
            off += ncols

    # encC packed: [128, sum_b nsc_b*1024]; col sc*1024+h, partition = s%128
    tot_c = sum(nsc * 1024 for nsc in nsc_b)
    encC = np.zeros((128, tot_c), np.float32)
    off = 0
    for b in range(B):
        for sc in range(nsc_b[b]):
            encC[:, off:off + 1024] = enc[b, sc * 128:(sc + 1) * 128, :]
            off += 1024
    del enc

    # gx: [T, B, 4096] for this core's slots
    gx = gx_full[bl].transpose(1, 0, 2)  # [T, B, 4096]

    rmask = np.zeros((B, B, 512), np.float32)
    for b in range(B):
        rmask[b, b, :] = 1.0
    rmask = rmask.transpose(1, 0, 2).reshape(B, B * 512)
    rmask40 = np.zeros((40, B * 512), np.float32)
    rmask40[0:B] = rmask
    rmask40[32:32 + B] = rmask
    return {
        "rmask": rmask40.astype(np.uint8),
        "encT": bf16(encT),
        "encC": bf16(encC),
        "gx": bf16(gx),
        "h0T": bf16(h0.T.reshape(NHC, 128, B)),
        "o0T": bf16(init_out.T.reshape(4, 128, B)),
        "c0": np.ascontiguousarray(c0),
        "valid": bf16(valid),
    }


def _prep_shared_weights(inputs):
    W_ih = np.asarray(inputs["W_ih"], np.float32)
    W_hh = np.asarray(inputs["W_hh"], np.float32)
    attn_W = np.asarray(inputs["attn_W"], np.float32)
    p1W = np.asarray(inputs["proj1_W"], np.float32)
    p2W = np.asarray(inputs["proj2_W"], np.float32)

    # recurrent weights [prev_out | h]: Wcat [4096, 1536] -> blocks kc [128,4096]
    Wcat = np.concatenate([W_ih[:, D:], W_hh], axis=1)  # [4H, 1536]
    blocks = Wcat.T.reshape(NKC, 128, 4096)  # kc-major
    wrec = np.zeros((4, 3, 128, 4096), np.float32)
    for q in range(4):
        for g in range(3):
            for j in range(4):
                wrec[q, g, :, j * 1024:(j + 1) * 1024] = (
                    blocks[g * 4 + j][:, q * 1024:(q + 1) * 1024]
                )
    # attn [2, 128, 4096]: tile ag, col j*1024+n = attn block (hc=ag*4+j)
    ablocks = np.ascontiguousarray(attn_W.T).reshape(NHC, 128, H)
    attn = np.zeros((2, 128, 4096), np.float32)
    for ag in range(2):
        for j in range(4):
            attn[ag, :, j * 1024:(j + 1) * 1024] = ablocks[ag * 4 + j]
    # p1 [4, 128, 4096]
    pblocks = p1W.T.reshape(16, 128, H)
    p1 = np.zeros((4, 128, 4096), np.float32)
    for pg in range(4):
        for j in range(4):
            p1[pg, :, j * 1024:(j + 1) * 1024] = pblocks[pg * 4 + j]
    return {
        "wrec": bf16(wrec),
        "attnW": bf16(attn),
        "p1T": bf16(p1),
        "p2T": bf16(p2W.T.reshape(NHC, 128, D)),
    }


def run(inputs, T=T_FULL, trace=False):
    slen_all = np.asarray(inputs["source_length"]).astype(np.int64)
    batch_lists, nsc_b, sL_b = _sorted_assignment(slen_all)
    nc = build_core_kernel(nsc_b, sL_b, T=T)

    # hoisted x-part of the gates for all (b, t): [64, 64, 4096] fp32
    target = np.asarray(inputs["target"], np.float32)
    Wx = np.asarray(inputs["W_ih"], np.float32)[:, :D]  # [4096, 512]
    gx_full = np.ascontiguousarray(
        np.tensordot(target[:, :T], Wx, axes=([2], [1]))
    )  # [64, T, 4096]

    shared = _prep_shared_weights(inputs)
    in_maps = []
    for c in range(NCORES):
        m = _prep_core_inputs(inputs, batch_lists[c], nsc_b, sL_b, gx_full, T=T)
        m.update(shared)
        in_maps.append(m)
    res = run_bass_kernel_spmd(nc, in_maps, core_ids=list(range(NCORES)), trace=trace)
    out = np.zeros((B_FULL, T, D), np.float32)
    for c in range(NCORES):
        o = res.results[c]["out"]
        for s in range(B):
            out[batch_lists[c][s]] = o[s]
    return out, res


def kernel(**inputs) -> np.ndarray:
    out, _ = run(inputs)
    return out


if __name__ == "__main__":
    np.random.seed(0)
    print("smoke build only")
    nc = build_core_kernel([5, 6, 6, 6, 7, 7, 8, 8], [611, 653, 706, 761, 822, 888, 955, 1018], T=2)
    print("build ok")
